# revision 14
# baseline (speedup 1.0000x reference)
"""NodeGraphContrastiveLoss on 8 Trainium2 cores.

loss = mean_n[ ln(rowsum_n - exp(z_pos_n)) - z_pos_n ],  z = cos(l_n, g_k)/T.

Sharding: rows of l2=[131072,256] split 8 ways (16384 rows/core = 128
tiles of 128). g ([1024,256]) replicated, rolled per-core so tile t's
positive graph sits at column t of the similarity tile.

Per tile [128 rows x 1024 graphs]:
  - 2 fp8(e4m3) DoubleRow matmuls (256-deep contraction in-instruction),
    psum dot = 64*z  (l rows scaled 4/(T*||l||), g rows 16/||g||, on host).
  - exp + row-sum split across two engines: even tiles on ScalarE
    (Exp activation, scale=1/64, fused accum), odd tiles on a custom DVE
    op computing ((1+t)^2+1)^8 = 256*exp(z), t = psum/512, fused accum.
The positives never touch the device: z_pos is recomputed exactly on host
(f64), ln(ep) = z_pos is analytic, and host-exp(z_pos) is subtracted from
the device row-sum (the quantization mismatch is ~1e-5 of the row-sum).
Final ln() and the mean over 131072 rows happen on host.
"""

from operator import add

import numpy as np
import ml_dtypes
from contextlib import ExitStack

import concourse.bass as bass
import concourse.tile as tile
from concourse import bacc, mybir, dve_ops
from concourse.bass_utils import run_bass_kernel_spmd
from concourse.dve_spec import Spec, Src0, C0, Zero, One, sq

T = 0.2
N_CORES = 8
B, A, C, K = 1024, 128, 256, 1024
N = B * A              # 131072 rows total
NL = N // N_CORES      # 16384 rows per core
NT = NL // 128         # 128 tiles per core
BLK = 4                # tiles per DMA block
SL = 4.0               # fp8 scale for l rows (applied after 1/(T*norm))
SG = 16.0              # fp8 scale for g rows (applied after 1/norm)
SP = SL * SG           # psum = SP * z
FP8 = ml_dtypes.float8_e4m3

F32 = mybir.dt.float32
E4M3 = mybir.dt.float8e4
BF = mybir.dt.bfloat16
AF = mybir.ActivationFunctionType

LAST_RESULTS = None  # BassKernelResults of the most recent run (for test.py)
_NC = None


def _exp8_ref(in0, in1, s0, s1, imm2):
    t = in0.astype(np.float32) * np.float32(s0)
    e = np.float32(1.0) + t
    q = e * e + np.float32(1.0)
    b = q * q
    b = b * b
    b = (b * b).astype(np.float32)
    return b, b.reshape(b.shape[0], -1).sum(axis=-1, keepdims=True)


def _register_exp_op():
    """((1 + in0*s0)^2 + 1)^8 = 256*exp(16*in0*s0) to ~Taylor-2-of-exp(x/8)
    accuracy, with accum_out = row sum. Registered once per process."""
    name = "EXP8_SUM_ANT"
    for op in dve_ops.OPS:
        if op.name == name:
            return op
    t = Src0 * C0
    q = sq(One + t) + One
    spec = Spec(
        body=sq(sq(sq(q))),
        accum=add,
        accum_init=Zero,
        reference=_exp8_ref,
    )
    op = dve_ops.DveOp(
        name,
        spec,
        subdim=False,
        uops_sha={"v3": "32c57a56fd8e20d2", "v4": "b219ed9b957dc2d8"},
    )
    dve_ops.OPS.append(op)
    dve_ops.CUSTOM_DVE_SPECS[name] = spec
    dve_ops._SUB_OPCODE_FOR_NAME[name] = (
        dve_ops._CUSTOM_DVE_ROW_BASE + len(dve_ops.OPS) - 1
    )
    assert dve_ops._SUB_OPCODE_FOR_NAME[name] < 0x20
    return op


EXP8_SUM = _register_exp_op()


def _build():
    nc = bacc.Bacc(None, target_bir_lowering=False)
    # lt[b, p, j, c, r] = l_q[row (b*BLK+j)*128 + r, channel c*128 + p]
    lt = nc.dram_tensor("lt", [NT // BLK, 128, BLK, 2, 128], E4M3,
                        kind="ExternalInput")
    # g[p, c, n] = g_q[graph n (rolled), channel c*128 + p]
    g = nc.dram_tensor("g", [128, 2, K], E4M3, kind="ExternalInput")
    rs_out = nc.dram_tensor("rs", [128, NT], F32, kind="ExternalOutput")

    with tile.TileContext(nc) as tc, ExitStack() as ctx:
        singles = ctx.enter_context(tc.tile_pool(name="singles", bufs=1))
        lt_pool = ctx.enter_context(tc.tile_pool(name="ltp", bufs=5))
        psum = ctx.enter_context(tc.tile_pool(name="psum", bufs=4, space="PSUM"))

        gh = singles.tile([128, 2, K], E4M3)
        nc.sync.dma_start(out=gh[:], in_=g[:, :, :])

        rowsum_all = singles.tile([128, NT], F32)

        # PE p-state warmup: dummy matmuls on zeroed SBUF while the first
        # input DMAs are in flight, so the real matmuls start at full clock.
        wk = singles.tile([128, 2, 128], E4M3)
        wr = singles.tile([128, 2, 512], E4M3)
        nc.vector.memset(wk[:], 0)
        nc.vector.memset(wr[:], 0)
        for _ in range(16):
            wp = psum.tile([128, K], F32, tag="ps")
            nc.tensor.matmul(
                wp[:, 0:512], wk[:], wr[:],
                start=True, stop=True,
                perf_mode=mybir.MatmulPerfMode.DoubleRow,
            )

        RS_CHUNK = 32
        for b in range(NT // BLK):
            cb = lt_pool.tile([128, BLK, 2, 128], E4M3, tag="cb")
            if b == 0:
                # per-tile DMAs for the first block: tile 0's matmul can
                # start ~1.2us earlier than with one 4-tile transfer
                for j in range(BLK):
                    nc.sync.dma_start(out=cb[:, j], in_=lt[0][:, j])
            else:
                nc.sync.dma_start(out=cb[:], in_=lt[b])
            for j in range(BLK):
                t = b * BLK + j
                ps = psum.tile([128, K], F32, tag="ps")
                for h in range(2):
                    nc.tensor.matmul(
                        ps[:, h * 512:(h + 1) * 512],
                        cb[:, j],
                        gh[:, :, h * 512:(h + 1) * 512],
                        start=True, stop=True,
                        perf_mode=mybir.MatmulPerfMode.DoubleRow,
                    )
                if t % 2 == 0:
                    nc.scalar.activation(
                        out=ps[:], in_=ps[:], func=AF.Exp,
                        scale=1.0 / SP,
                        accum_out=rowsum_all[:, t:t + 1],
                    )
                else:
                    nc.vector._custom_dve(
                        EXP8_SUM,
                        out=ps[:], in0=ps[:],
                        s0=1.0 / (SP * 8.0),
                        accum_out=rowsum_all[:, t:t + 1],
                    )
                # stream the row-sums out in chunks to hide the DMA tail
                if (t + 1) % RS_CHUNK == 0:
                    c0 = t + 1 - RS_CHUNK
                    nc.sync.dma_start(out=rs_out[:, c0:t + 1],
                                      in_=rowsum_all[:, c0:t + 1])
    nc.finalize()
    return nc


def _get_nc():
    global _NC
    if _NC is None:
        _NC = _build()
    return _NC


def _prep_core(lq, g_q, i):
    # lq: [N, 256] fp8 (already scaled); slice this core's rows and
    # transpose to [blocks, chan_lo(part), tile, chan_hi, row].
    rows = lq[i * NL:(i + 1) * NL]
    lt5 = rows.reshape(NT // BLK, BLK, 128, 2, 128)        # [b, j, r, c, p]
    ltT = np.ascontiguousarray(lt5.transpose(0, 4, 1, 3, 2))
    gr = np.roll(g_q, -i * A, axis=0)                      # [K, 256]
    ghT = np.ascontiguousarray(
        gr.T.reshape(2, 128, K).transpose(1, 0, 2))        # [p, c, K]
    return {"lt": ltT, "g": ghT}


def kernel(l_enc, g_enc, **run_kwargs):
    global LAST_RESULTS
    l2 = np.asarray(l_enc, dtype=np.float32).reshape(N, C)
    ge = np.asarray(g_enc, dtype=np.float32)

    lnorm = np.sqrt(np.einsum("nc,nc->n", l2, l2))
    lq = (l2 * (SL / (T * lnorm))[:, None]).astype(FP8)
    gnorm = np.sqrt(np.einsum("kc,kc->k", ge, ge))
    gq = (ge * (SG / gnorm)[:, None]).astype(FP8)

    # exact positive logits on host: z_pos[n] = cos(l_n, g_{n//A}) / T
    zpos = (
        np.einsum("krc,kc->kr", l2.reshape(K, A, C), ge / gnorm[:, None])
        .reshape(N) / (T * lnorm)
    )

    in_maps = [_prep_core(lq, gq, i) for i in range(N_CORES)]
    nc = _get_nc()
    res = run_bass_kernel_spmd(nc, in_maps, core_ids=list(range(N_CORES)),
                               **run_kwargs)
    LAST_RESULTS = res

    # rowsum scale: even tiles (ScalarE) exact, odd tiles (DVE op) x256
    rs_scale = np.where(np.arange(NT) % 2 == 0, 1.0, 1.0 / 256.0)
    total = 0.0
    for i, r in enumerate(res.results):
        rs = np.asarray(r["rs"], dtype=np.float64) * rs_scale[None, :]
        # rs[p, t] is the row-sum of global row i*NL + t*128 + p
        zp = zpos[i * NL:(i + 1) * NL].reshape(NT, 128).T
        total += float(np.sum(np.log(rs - np.exp(zp)) - zp))
    return np.float32(total / N)


# revision 17
# speedup vs baseline: 1.0107x; 1.0107x over previous
"""NodeGraphContrastiveLoss on 8 Trainium2 cores.

loss = mean_n[ ln(rowsum_n - exp(z_pos_n)) - z_pos_n ],  z = cos(l_n, g_k)/T.

Sharding: rows of l2=[131072,256] split 8 ways (16384 rows/core = 128
tiles of 128). g ([1024,256]) replicated, rolled per-core so tile t's
positive graph sits at column t of the similarity tile.

Per tile [128 rows x 1024 graphs]:
  - 2 fp8(e4m3) DoubleRow matmuls (256-deep contraction in-instruction),
    psum dot = 64*z  (l rows scaled 4/(T*||l||), g rows 16/||g||, on host).
  - exp + row-sum split across two engines: even tiles on ScalarE
    (Exp activation, scale=1/64, fused accum), odd tiles on a custom DVE
    op computing ((1+t)^2+1)^8 = 256*exp(z), t = psum/512, fused accum.
The positives never touch the device: z_pos is recomputed exactly on host
(f64), ln(ep) = z_pos is analytic, and host-exp(z_pos) is subtracted from
the device row-sum (the quantization mismatch is ~1e-5 of the row-sum).
Final ln() and the mean over 131072 rows happen on host.
"""

from operator import add

import numpy as np
import ml_dtypes
from contextlib import ExitStack

import concourse.bass as bass
import concourse.tile as tile
from concourse import bacc, mybir, dve_ops
from concourse.bass_utils import run_bass_kernel_spmd
from concourse.dve_spec import Spec, Src0, C0, Zero, One, sq

T = 0.2
N_CORES = 8
B, A, C, K = 1024, 128, 256, 1024
N = B * A              # 131072 rows total
NL = N // N_CORES      # 16384 rows per core
NT = NL // 128         # 128 tiles per core
BLK = 4                # tiles per DMA block
SL = 4.0               # fp8 scale for l rows (applied after 1/(T*norm))
SG = 16.0              # fp8 scale for g rows (applied after 1/norm)
SP = SL * SG           # psum = SP * z
FP8 = ml_dtypes.float8_e4m3

F32 = mybir.dt.float32
E4M3 = mybir.dt.float8e4
BF = mybir.dt.bfloat16
AF = mybir.ActivationFunctionType

LAST_RESULTS = None  # BassKernelResults of the most recent run (for test.py)
_NC = None


def _exp8_ref(in0, in1, s0, s1, imm2):
    t = in0.astype(np.float32) * np.float32(s0)
    e = np.float32(1.0) + t
    q = e * e + np.float32(1.0)
    b = q * q
    b = b * b
    b = (b * b).astype(np.float32)
    return b, b.reshape(b.shape[0], -1).sum(axis=-1, keepdims=True)


def _register_exp_op():
    """((1 + in0*s0)^2 + 1)^8 = 256*exp(16*in0*s0) to ~Taylor-2-of-exp(x/8)
    accuracy, with accum_out = row sum. Registered once per process."""
    name = "EXP8_SUM_ANT"
    for op in dve_ops.OPS:
        if op.name == name:
            return op
    t = Src0 * C0
    q = sq(One + t) + One
    spec = Spec(
        body=sq(sq(sq(q))),
        accum=add,
        accum_init=Zero,
        reference=_exp8_ref,
    )
    op = dve_ops.DveOp(
        name,
        spec,
        subdim=False,
        uops_sha={"v3": "32c57a56fd8e20d2", "v4": "b219ed9b957dc2d8"},
    )
    dve_ops.OPS.append(op)
    dve_ops.CUSTOM_DVE_SPECS[name] = spec
    dve_ops._SUB_OPCODE_FOR_NAME[name] = (
        dve_ops._CUSTOM_DVE_ROW_BASE + len(dve_ops.OPS) - 1
    )
    assert dve_ops._SUB_OPCODE_FOR_NAME[name] < 0x20
    return op


EXP8_SUM = _register_exp_op()


ACT_PARITY = 0     # tiles with t % 2 == ACT_PARITY go to ScalarE
WARMUP = 8         # dummy PE matmuls before the main loop
LT_BUFS = 4
RS_CHUNK = 16
SPLIT_FIRST = False


def _build():
    nc = bacc.Bacc(None, target_bir_lowering=False)
    # lt[b, p, j, c, r] = l_q[row (b*BLK+j)*128 + r, channel c*128 + p]
    lt = nc.dram_tensor("lt", [NT // BLK, 128, BLK, 2, 128], E4M3,
                        kind="ExternalInput")
    # g[p, c, n] = g_q[graph n (rolled), channel c*128 + p]
    g = nc.dram_tensor("g", [128, 2, K], E4M3, kind="ExternalInput")
    rs_out = nc.dram_tensor("rs", [128, NT], F32, kind="ExternalOutput")

    with tile.TileContext(nc) as tc, ExitStack() as ctx:
        singles = ctx.enter_context(tc.tile_pool(name="singles", bufs=1))
        lt_pool = ctx.enter_context(tc.tile_pool(name="ltp", bufs=LT_BUFS))
        psum = ctx.enter_context(tc.tile_pool(name="psum", bufs=4, space="PSUM"))

        gh = singles.tile([128, 2, K], E4M3)
        nc.sync.dma_start(out=gh[:], in_=g[:, :, :])

        rowsum_all = singles.tile([128, NT], F32)

        if WARMUP:
            # PE p-state warmup while the first input DMAs are in flight
            wk = singles.tile([128, 2, 128], E4M3)
            wr = singles.tile([128, 2, 512], E4M3)
            nc.vector.memset(wk[:], 0)
            nc.vector.memset(wr[:], 0)
            for _ in range(WARMUP):
                wp = psum.tile([128, K], F32, tag="ps")
                nc.tensor.matmul(
                    wp[:, 0:512], wk[:], wr[:],
                    start=True, stop=True,
                    perf_mode=mybir.MatmulPerfMode.DoubleRow,
                )

        for b in range(NT // BLK):
            cb = lt_pool.tile([128, BLK, 2, 128], E4M3, tag="cb")
            if b == 0 and SPLIT_FIRST:
                # per-tile DMAs for the first block: tile 0's matmul can
                # start ~1.2us earlier than with one 4-tile transfer
                for j in range(BLK):
                    nc.sync.dma_start(out=cb[:, j], in_=lt[0][:, j])
            else:
                nc.sync.dma_start(out=cb[:], in_=lt[b])
            for j in range(BLK):
                t = b * BLK + j
                ps = psum.tile([128, K], F32, tag="ps")
                for h in range(2):
                    nc.tensor.matmul(
                        ps[:, h * 512:(h + 1) * 512],
                        cb[:, j],
                        gh[:, :, h * 512:(h + 1) * 512],
                        start=True, stop=True,
                        perf_mode=mybir.MatmulPerfMode.DoubleRow,
                    )
                if t % 2 == ACT_PARITY:
                    nc.scalar.activation(
                        out=ps[:], in_=ps[:], func=AF.Exp,
                        scale=1.0 / SP,
                        accum_out=rowsum_all[:, t:t + 1],
                    )
                else:
                    nc.vector._custom_dve(
                        EXP8_SUM,
                        out=ps[:], in0=ps[:],
                        s0=1.0 / (SP * 8.0),
                        accum_out=rowsum_all[:, t:t + 1],
                    )
                # stream the row-sums out in chunks to hide the DMA tail
                if (t + 1) % RS_CHUNK == 0:
                    c0 = t + 1 - RS_CHUNK
                    nc.sync.dma_start(out=rs_out[:, c0:t + 1],
                                      in_=rowsum_all[:, c0:t + 1])
    nc.finalize()
    return nc


def _get_nc():
    global _NC
    if _NC is None:
        _NC = _build()
    return _NC


def _prep_core(lq, g_q, i):
    # lq: [N, 256] fp8 (already scaled); slice this core's rows and
    # transpose to [blocks, chan_lo(part), tile, chan_hi, row].
    rows = lq[i * NL:(i + 1) * NL]
    lt5 = rows.reshape(NT // BLK, BLK, 128, 2, 128)        # [b, j, r, c, p]
    ltT = np.ascontiguousarray(lt5.transpose(0, 4, 1, 3, 2))
    gr = np.roll(g_q, -i * A, axis=0)                      # [K, 256]
    ghT = np.ascontiguousarray(
        gr.T.reshape(2, 128, K).transpose(1, 0, 2))        # [p, c, K]
    return {"lt": ltT, "g": ghT}


def kernel(l_enc, g_enc, **run_kwargs):
    global LAST_RESULTS
    l2 = np.asarray(l_enc, dtype=np.float32).reshape(N, C)
    ge = np.asarray(g_enc, dtype=np.float32)

    lnorm = np.sqrt(np.einsum("nc,nc->n", l2, l2))
    lq = (l2 * (SL / (T * lnorm))[:, None]).astype(FP8)
    gnorm = np.sqrt(np.einsum("kc,kc->k", ge, ge))
    gq = (ge * (SG / gnorm)[:, None]).astype(FP8)

    # exact positive logits on host: z_pos[n] = cos(l_n, g_{n//A}) / T
    zpos = (
        np.einsum("krc,kc->kr", l2.reshape(K, A, C), ge / gnorm[:, None])
        .reshape(N) / (T * lnorm)
    )

    in_maps = [_prep_core(lq, gq, i) for i in range(N_CORES)]
    nc = _get_nc()
    res = run_bass_kernel_spmd(nc, in_maps, core_ids=list(range(N_CORES)),
                               **run_kwargs)
    LAST_RESULTS = res

    # ScalarE tiles exact, DVE-op tiles x256
    rs_scale = np.where(np.arange(NT) % 2 == ACT_PARITY, 1.0, 1.0 / 256.0)
    total = 0.0
    for i, r in enumerate(res.results):
        rs = np.asarray(r["rs"], dtype=np.float64) * rs_scale[None, :]
        # rs[p, t] is the row-sum of global row i*NL + t*128 + p
        zp = zpos[i * NL:(i + 1) * NL].reshape(NT, 128).T
        total += float(np.sum(np.log(rs - np.exp(zp)) - zp))
    return np.float32(total / N)
